# revision 11
# baseline (speedup 1.0000x reference)
"""Single-head attention (B=8, S=2048, H=768, D=64) on 8 TRN2 NeuronCores.

Strategy: data-parallel over batch — core b computes batch element b end to
end; no collectives. Host pre-transposes Q/K/V to [H, S] bf16 so every
matmul contraction lands on the partition axis with no device transposes.

Per-core dataflow (all matmuls bf16 x bf16 -> f32 PSUM):
  qT[d,s] = Wq^T @ queryT   (lhsT=Wq tile [128,64], rhs=queryT tile)
  kT[d,s], vT[d,s] likewise; bias added per-partition on PSUM->SBUF copy.
  vT is PE-transposed back to V[sk,d] tiles and packed with a ones column
  into V_ext[sk,65] so the softmax denominator falls out of the AV matmul.
  S^T[sk,sq] = kT.T @ qT    (contraction d=64 on partitions)
  P^T = exp(S^T/8 + mask_bias)   mask bias is per-partition (sk) in this
                                  layout -> fused into the Exp activation.
  O_ext^T[65,sq] = sum_sk V_ext^T @ P^T   (PSUM accumulation over sk tiles)
  out[sq,d] = PE-transpose(O_ext^T) row-scaled by 1/denom.
"""

import os
from contextlib import ExitStack

import numpy as np
import ml_dtypes

import concourse.bass as bass
import concourse.mybir as mybir
import concourse.tile as tile
from concourse import bacc
from concourse.bass_utils import run_bass_kernel_spmd
from concourse.masks import make_identity

S, H, D = 2048, 768, 64
P = 128
NT = S // P      # 16 sk tiles
HT = H // P      # 6 h tiles
CH = 512         # sq chunk for matmul free dim (PSUM bank)
NCH = S // CH    # 4
BF = mybir.dt.bfloat16
F32 = mybir.dt.float32
AF = mybir.ActivationFunctionType

LAST_RESULT = None  # BassKernelResults of the most recent run (for test.py)


def _build():
    nc = bacc.Bacc()
    qT_d = nc.declare_dram_parameter("qT", [H, S], BF, isOutput=False)
    kT_d = nc.declare_dram_parameter("kT", [H, S], BF, isOutput=False)
    vT_d = nc.declare_dram_parameter("vT", [H, S], BF, isOutput=False)
    w_d = {
        "q": nc.declare_dram_parameter("wq", [H, D], BF, isOutput=False),
        "k": nc.declare_dram_parameter("wk", [H, D], BF, isOutput=False),
        "v": nc.declare_dram_parameter("wv", [H, D], BF, isOutput=False),
    }
    b_d = {
        "q": nc.declare_dram_parameter("bq", [D, 1], F32, isOutput=False),
        "k": nc.declare_dram_parameter("bk", [D, 1], F32, isOutput=False),
        "v": nc.declare_dram_parameter("bv", [D, 1], F32, isOutput=False),
    }
    mb_d = nc.declare_dram_parameter("mb", [P, NT], F32, isOutput=False)
    o_d = nc.declare_dram_parameter("o", [S, D], F32, isOutput=True)
    in_d = {"q": qT_d, "k": kT_d, "v": vT_d}

    with ExitStack() as ctx:
        tc = ctx.enter_context(tile.TileContext(nc))
        consts = ctx.enter_context(tc.tile_pool(name="consts", bufs=1))
        stage = ctx.enter_context(tc.tile_pool(name="stage", bufs=3 * HT))
        persist = ctx.enter_context(tc.tile_pool(name="persist", bufs=1))
        ppool = ctx.enter_context(tc.tile_pool(name="ppool", bufs=4))
        ostage = ctx.enter_context(tc.tile_pool(name="ostage", bufs=3))
        small = ctx.enter_context(tc.tile_pool(name="small", bufs=3))
        psw = ctx.enter_context(tc.tile_pool(name="psw", bufs=2, space="PSUM"))
        pso = ctx.enter_context(tc.tile_pool(name="pso", bufs=1, space="PSUM"))

        # ---- constants ----
        w_sb = consts.tile([P, 3, HT, D], BF, tag="w")  # wq|wk|wv h-tiles
        for i, t in enumerate("qkv"):
            nc.sync.dma_start(
                out=w_sb[:, i, :, :],
                in_=w_d[t][:, :].rearrange("(t p) n -> p t n", p=P),
            )
        b_sb = consts.tile([D, 3], F32, tag="b")
        for i, t in enumerate("qkv"):
            nc.sync.dma_start(out=b_sb[:, i : i + 1], in_=b_d[t][:, :])
        mb_sb = consts.tile([P, NT], F32, tag="mb")
        nc.sync.dma_start(out=mb_sb[:, :], in_=mb_d[:, :])
        ident = consts.tile([P, P], F32, tag="ident")
        make_identity(nc, ident)
        ident_bf = consts.tile([P, P], BF, tag="ident_bf")
        make_identity(nc, ident_bf)

        # ---- persistent SBUF tensors ----
        qT_sb = persist.tile([D, S], BF, tag="qT")
        kT_sb = persist.tile([D, S], BF, tag="kT")
        vT_sb = persist.tile([D, S], BF, tag="vT")
        proj_sb = {"q": qT_sb, "k": kT_sb, "v": vT_sb}
        vE_sb = persist.tile([P, NT * (D + 1)], BF, tag="vE")  # V_ext tiles
        nc.vector.memset(vE_sb, 1.0)  # ones column (col 64 of each tile)
        oT_sb = persist.tile([D + 1, S], F32, tag="oT")  # O_ext^T staging

        # ---- input load: all 18 h-tiles land in fresh slots (no slot
        # recycling -> each HWDGE DMA carries zero wait commands) ----
        st_all = {}
        for t in "qkv":
            for h in range(HT):
                st = stage.tile(
                    [P, S], BF, tag="stage", name=f"st_{t}{h}"
                )
                nc.sync.dma_start(out=st, in_=in_d[t][h * P : (h + 1) * P, :])
                st_all[t, h] = st

        # ---- projections: xT[d, s] = W^T @ inputT, h-outer c-inner;
        # the 4 chunk accumulators temporally share the O PSUM banks ----
        for i, t in enumerate("qkv"):
            pp = [
                pso.tile([D, CH], F32, tag=f"o{c}", name=f"pp_{t}{c}")
                for c in range(NCH)
            ]
            for h in range(HT):
                for c in range(NCH):
                    nc.tensor.matmul(
                        pp[c],
                        lhsT=w_sb[:, i, h, :],
                        rhs=st_all[t, h][:, c * CH : (c + 1) * CH],
                        start=(h == 0),
                        stop=(h == HT - 1),
                    )
            for c in range(NCH):
                # PSUM -> SBUF with per-partition (d) bias add
                nc.vector.tensor_scalar_add(
                    out=proj_sb[t][:, c * CH : (c + 1) * CH],
                    in0=pp[c],
                    scalar1=b_sb[:, i : i + 1],
                )

        # ---- V_ext: PE-transpose vT [64, 128] tiles -> [128, 64] ----
        for k in range(NT):
            pt = psw.tile([P, D], BF, tag="work")
            nc.tensor.transpose(
                pt,
                in_=vT_sb[:, k * P : (k + 1) * P],
                identity=ident_bf[:D, :D],
            )
            nc.vector.tensor_copy(
                out=vE_sb[:, k * (D + 1) : k * (D + 1) + D], in_=pt
            )

        # ---- attention: scores^T, exp, AV accumulate ----
        po = [
            pso.tile([D + 1, CH], F32, tag=f"o{c}", name=f"po{c}")
            for c in range(NCH)
        ]
        for k in range(NT):
            pT = ppool.tile([P, S], BF, tag="pT", name=f"pT{k}")
            for half in range(2):
                ps = psw.tile([P, 2 * CH], F32, tag="work")
                for j in range(2):
                    sq0 = half * 2 * CH + j * CH
                    nc.tensor.matmul(
                        ps[:, j * CH : (j + 1) * CH],
                        lhsT=kT_sb[:, k * P : (k + 1) * P],
                        rhs=qT_sb[:, sq0 : sq0 + CH],
                        start=True,
                        stop=True,
                    )
                # P^T = exp(S^T/8 + mask_bias), straight into bf16 SBUF
                nc.scalar.activation(
                    out=pT[:, half * 2 * CH : (half + 1) * 2 * CH],
                    in_=ps,
                    func=AF.Exp,
                    bias=mb_sb[:, k : k + 1],
                    scale=0.125,
                )
            for c in range(NCH):
                nc.tensor.matmul(
                    po[c],
                    lhsT=vE_sb[:, k * (D + 1) : (k + 1) * (D + 1)],
                    rhs=pT[:, c * CH : (c + 1) * CH],
                    start=(k == 0),
                    stop=(k == NT - 1),
                )

        # ---- epilogue: normalize + transpose to [sq, d] ----
        for c in range(NCH):
            nc.vector.tensor_copy(
                out=oT_sb[:, c * CH : (c + 1) * CH], in_=po[c]
            )
        for k in range(NT):
            pt = psw.tile([P, D + 1], F32, tag="work")
            nc.tensor.transpose(
                pt,
                in_=oT_sb[:, k * P : (k + 1) * P],
                identity=ident[: D + 1, : D + 1],
            )
            r = small.tile([P, 1], F32, tag="recip")
            nc.vector.reciprocal(r, pt[:, D : D + 1])
            ot = ostage.tile([P, D], F32, tag="ot")
            nc.vector.tensor_scalar_mul(ot, pt[:, :D], r)
            nc.sync.dma_start(out=o_d[k * P : (k + 1) * P, :], in_=ot)

    return nc


_NC = None


def kernel(query, key, value, mask, Wq, bq, Wk, bk, Wv, bv):
    global _NC, LAST_RESULT
    bf16 = ml_dtypes.bfloat16
    B = query.shape[0]
    assert B == 8

    if _NC is None:
        _NC = _build()
        _NC.finalize()  # run bacc passes (wait splitting, reg alloc, ACT tables)

    wqb = np.ascontiguousarray(np.asarray(Wq).astype(bf16))
    wkb = np.ascontiguousarray(np.asarray(Wk).astype(bf16))
    wvb = np.ascontiguousarray(np.asarray(Wv).astype(bf16))
    bqf = np.asarray(bq, np.float32).reshape(D, 1)
    bkf = np.asarray(bk, np.float32).reshape(D, 1)
    bvf = np.asarray(bv, np.float32).reshape(D, 1)

    in_maps = []
    for b in range(B):
        mb = ((np.asarray(mask[b], np.float32) - 1.0) * 1e9).reshape(NT, P).T
        in_maps.append(
            {
                "qT": np.ascontiguousarray(np.asarray(query[b]).T.astype(bf16)),
                "kT": np.ascontiguousarray(np.asarray(key[b]).T.astype(bf16)),
                "vT": np.ascontiguousarray(np.asarray(value[b]).T.astype(bf16)),
                "wq": wqb,
                "wk": wkb,
                "wv": wvb,
                "bq": bqf,
                "bk": bkf,
                "bv": bvf,
                "mb": np.ascontiguousarray(mb),
            }
        )

    res = run_bass_kernel_spmd(
        _NC,
        in_maps,
        core_ids=list(range(8)),
        trace=bool(os.environ.get("KERNEL_TRACE")),
    )
    LAST_RESULT = res
    out = np.stack([np.asarray(res.results[i]["o"]) for i in range(B)])
    return out.astype(np.float32)


# revision 12
# speedup vs baseline: 1.2694x; 1.2694x over previous
"""Single-head attention (B=8, S=2048, H=768, D=64) on 8 TRN2 NeuronCores.

Strategy: data-parallel over batch — core b computes batch element b end to
end; no collectives. Host pre-transposes Q/K/V to [H, S] bf16 so every
matmul contraction lands on the partition axis with no device transposes.

Per-core dataflow (all matmuls bf16 x bf16 -> f32 PSUM):
  [qT; kT][128, s] = [Wq|Wk]^T @ [queryT, keyT]  via col-packed matmuls
    (array cols 0-63 run the q projection, 64-127 the k projection,
     concurrently).  v projection packs chunk pairs the same way.
  kT / qT are then partition-duplicated (SBUF->SBUF DMA) into kkT/qqT so
  scores can row-pack: array rows 0-63 compute sk-tile 2j while rows
  64-127 compute sk-tile 2j+1 concurrently (contraction d=64 per group).
  P^T = exp(S^T/8 + mask_bias)   mask bias is per-partition (sk) in this
                                  layout -> fused into the Exp activation.
  O_ext^T[65,sq] = sum_sk V_ext^T @ P^T   (V_ext has a ones column so the
                                  softmax denominator falls out of the AV
                                  matmul as row 64)
  out[sq,d] = PE-transpose(O_ext^T) row-scaled by 1/denom.
"""

import os
from contextlib import ExitStack

import numpy as np
import ml_dtypes

import concourse.bass as bass
import concourse.mybir as mybir
import concourse.tile as tile
from concourse import bacc
from concourse.bass_utils import run_bass_kernel_spmd
from concourse.masks import make_identity

S, H, D = 2048, 768, 64
P = 128
NT = S // P      # 16 sk tiles
HT = H // P      # 6 h tiles
CH = 512         # sq chunk for matmul free dim (PSUM bank)
NCH = S // CH    # 4
BF = mybir.dt.bfloat16
F32 = mybir.dt.float32
AF = mybir.ActivationFunctionType

LAST_RESULT = None  # BassKernelResults of the most recent run (for test.py)


def _build():
    nc = bacc.Bacc()
    qT_d = nc.declare_dram_parameter("qT", [H, S], BF, isOutput=False)
    kT_d = nc.declare_dram_parameter("kT", [H, S], BF, isOutput=False)
    vT_d = nc.declare_dram_parameter("vT", [H, S], BF, isOutput=False)
    wqk_d = nc.declare_dram_parameter("wqk", [H, P], BF, isOutput=False)
    wvv_d = nc.declare_dram_parameter("wvv", [H, P], BF, isOutput=False)
    bqk_d = nc.declare_dram_parameter("bqk", [P, 1], F32, isOutput=False)
    bvv_d = nc.declare_dram_parameter("bvv", [P, 1], F32, isOutput=False)
    mb_d = nc.declare_dram_parameter("mb", [P, NT], F32, isOutput=False)
    o_d = nc.declare_dram_parameter("o", [S, D], F32, isOutput=True)
    in_d = {"q": qT_d, "k": kT_d, "v": vT_d}

    with ExitStack() as ctx:
        tc = ctx.enter_context(tile.TileContext(nc))
        consts = ctx.enter_context(tc.tile_pool(name="consts", bufs=1))
        stage = ctx.enter_context(tc.tile_pool(name="stage", bufs=3 * HT))
        persist = ctx.enter_context(tc.tile_pool(name="persist", bufs=1))
        ppool = ctx.enter_context(tc.tile_pool(name="ppool", bufs=4))
        ostage = ctx.enter_context(tc.tile_pool(name="ostage", bufs=3))
        small = ctx.enter_context(tc.tile_pool(name="small", bufs=3))
        psw = ctx.enter_context(tc.tile_pool(name="psw", bufs=2, space="PSUM"))
        pso = ctx.enter_context(tc.tile_pool(name="pso", bufs=1, space="PSUM"))

        # ---- weights + input load first so transfers start immediately;
        # every tile lands in a fresh slot (no recycling -> no HWDGE waits)
        w_sb = consts.tile([P, 2, HT, P], BF, tag="w")  # wqk | wvv h-tiles
        nc.sync.dma_start(
            out=w_sb[:, 0, :, :],
            in_=wqk_d[:, :].rearrange("(t p) n -> p t n", p=P),
        )
        nc.sync.dma_start(
            out=w_sb[:, 1, :, :],
            in_=wvv_d[:, :].rearrange("(t p) n -> p t n", p=P),
        )
        st_all = {}
        for t in "qkv":
            for h in range(HT):
                st = stage.tile([P, S], BF, tag="stage", name=f"st_{t}{h}")
                nc.sync.dma_start(out=st, in_=in_d[t][h * P : (h + 1) * P, :])
                st_all[t, h] = st

        # ---- other constants ----
        bqk_sb = consts.tile([P, 1], F32, tag="bqk")
        nc.sync.dma_start(out=bqk_sb, in_=bqk_d[:, :])
        bvv_sb = consts.tile([P, 1], F32, tag="bvv")
        nc.sync.dma_start(out=bvv_sb, in_=bvv_d[:, :])
        mb_sb = consts.tile([P, NT], F32, tag="mb")
        nc.sync.dma_start(out=mb_sb, in_=mb_d[:, :])
        ident = consts.tile([P, P], F32, tag="ident")
        make_identity(nc, ident)
        ident_bf = consts.tile([P, P], BF, tag="ident_bf")
        make_identity(nc, ident_bf)

        # ---- persistent SBUF tensors ----
        qqT_sb = persist.tile([P, S], BF, tag="qqT")  # qT in both halves
        kkT_sb = persist.tile([P, S], BF, tag="kkT")  # kT in both halves
        vT2_sb = persist.tile([P, S // 2], BF, tag="vT2")  # vT chunk pairs
        vE_sb = persist.tile([P, NT * (D + 1)], BF, tag="vE")  # V_ext tiles
        nc.vector.memset(vE_sb, 1.0)  # ones column (col 64 of each tile)
        oT_sb = persist.tile([D + 1, S], F32, tag="oT")  # O_ext^T staging

        # ---- q/k projections, col-packed: psum rows 0:64 <- Wq^T @ queryT
        # (array cols 0-63), rows 64:128 <- Wk^T @ keyT (cols 64-127) ----
        for c in range(NCH):
            pp = pso.tile([P, CH], F32, tag=f"o{c}", name=f"pp{c}")
            for h in range(HT):
                nc.tensor.matmul(
                    pp[:D, :],
                    lhsT=w_sb[:, 0, h, :D],
                    rhs=st_all["q", h][:, c * CH : (c + 1) * CH],
                    start=(h == 0),
                    stop=(h == HT - 1),
                    tile_position=(0, 0),
                )
                nc.tensor.matmul(
                    pp[D:, :],
                    lhsT=w_sb[:, 0, h, D:],
                    rhs=st_all["k", h][:, c * CH : (c + 1) * CH],
                    start=(h == 0),
                    stop=(h == HT - 1),
                    tile_position=(0, D),
                )
            nc.vector.tensor_scalar_add(
                out=qqT_sb[:, c * CH : (c + 1) * CH], in0=pp, scalar1=bqk_sb
            )

        # ---- v projection, chunk pairs packed: rows 0:64 <- chunk 2u,
        # rows 64:128 <- chunk 2u+1 ----
        for u in range(NCH // 2):
            pv = pso.tile([P, CH], F32, tag=f"o{2 * u}", name=f"pv{u}")
            for h in range(HT):
                nc.tensor.matmul(
                    pv[:D, :],
                    lhsT=w_sb[:, 1, h, :D],
                    rhs=st_all["v", h][:, (2 * u) * CH : (2 * u + 1) * CH],
                    start=(h == 0),
                    stop=(h == HT - 1),
                    tile_position=(0, 0),
                )
                nc.tensor.matmul(
                    pv[D:, :],
                    lhsT=w_sb[:, 1, h, D:],
                    rhs=st_all["v", h][:, (2 * u + 1) * CH : (2 * u + 2) * CH],
                    start=(h == 0),
                    stop=(h == HT - 1),
                    tile_position=(0, D),
                )
            nc.vector.tensor_scalar_add(
                out=vT2_sb[:, u * CH : (u + 1) * CH], in0=pv, scalar1=bvv_sb
            )

        # ---- partition duplication via SBUF->SBUF DMA:
        # kkT = [kT; kT], then qqT's upper half kT -> overwritten with qT ----
        nc.sync.dma_start(out=kkT_sb[:D, :], in_=qqT_sb[D:, :])
        nc.sync.dma_start(out=kkT_sb[D:, :], in_=qqT_sb[D:, :])
        nc.sync.dma_start(out=qqT_sb[D:, :], in_=qqT_sb[:D, :])

        # ---- V_ext: PE-transpose vT [64, 128] pieces -> [128, 64] ----
        for k in range(NT):
            c = k // 4  # original chunk index
            base = (c // 2) * CH + (k % 4) * P
            lo, hi = (0, D) if c % 2 == 0 else (D, P)
            pt = psw.tile([P, D], BF, tag="work", name=f"ptv{k}")
            nc.tensor.transpose(
                pt,
                in_=vT2_sb[lo:hi, base : base + P],
                identity=ident_bf[lo:hi, lo:hi],
            )
            nc.vector.tensor_copy(
                out=vE_sb[:, k * (D + 1) : k * (D + 1) + D], in_=pt
            )

        # ---- attention: row-packed scores (sk-tile pair 2j / 2j+1 run on
        # array rows 0-63 / 64-127 concurrently), exp, AV accumulate ----
        po = [
            pso.tile([D + 1, CH], F32, tag=f"o{c}", name=f"po{c}")
            for c in range(NCH)
        ]
        for j in range(NT // 2):
            ta, tb = 2 * j, 2 * j + 1
            pTa = ppool.tile([P, S], BF, tag="pT", name=f"pTa{j}")
            pTb = ppool.tile([P, S], BF, tag="pT", name=f"pTb{j}")
            for half in range(2):
                ps_a = psw.tile([P, 2 * CH], F32, tag="work", name=f"psa{j}{half}")
                ps_b = psw.tile([P, 2 * CH], F32, tag="work", name=f"psb{j}{half}")
                for sub in range(2):
                    c = 2 * half + sub
                    nc.tensor.matmul(
                        ps_a[:, sub * CH : (sub + 1) * CH],
                        lhsT=kkT_sb[:D, ta * P : (ta + 1) * P],
                        rhs=qqT_sb[:D, c * CH : (c + 1) * CH],
                        start=True,
                        stop=True,
                        tile_position=(0, 0),
                    )
                    nc.tensor.matmul(
                        ps_b[:, sub * CH : (sub + 1) * CH],
                        lhsT=kkT_sb[D:, tb * P : (tb + 1) * P],
                        rhs=qqT_sb[D:, c * CH : (c + 1) * CH],
                        start=True,
                        stop=True,
                        tile_position=(D, 0),
                    )
                nc.scalar.activation(
                    out=pTa[:, half * 2 * CH : (half + 1) * 2 * CH],
                    in_=ps_a,
                    func=AF.Exp,
                    bias=mb_sb[:, ta : ta + 1],
                    scale=0.125,
                )
                nc.scalar.activation(
                    out=pTb[:, half * 2 * CH : (half + 1) * 2 * CH],
                    in_=ps_b,
                    func=AF.Exp,
                    bias=mb_sb[:, tb : tb + 1],
                    scale=0.125,
                )
            for k, pT in ((ta, pTa), (tb, pTb)):
                for c in range(NCH):
                    nc.tensor.matmul(
                        po[c],
                        lhsT=vE_sb[:, k * (D + 1) : (k + 1) * (D + 1)],
                        rhs=pT[:, c * CH : (c + 1) * CH],
                        start=(k == 0),
                        stop=(k == NT - 1),
                    )

        # ---- epilogue: normalize + transpose to [sq, d] ----
        for c in range(NCH):
            nc.vector.tensor_copy(out=oT_sb[:, c * CH : (c + 1) * CH], in_=po[c])
        for k in range(NT):
            pt = psw.tile([P, D + 1], F32, tag="work", name=f"pto{k}")
            nc.tensor.transpose(
                pt,
                in_=oT_sb[:, k * P : (k + 1) * P],
                identity=ident[: D + 1, : D + 1],
            )
            r = small.tile([P, 1], F32, tag="recip", name=f"r{k}")
            nc.vector.reciprocal(r, pt[:, D : D + 1])
            ot = ostage.tile([P, D], F32, tag="ot", name=f"ot{k}")
            nc.vector.tensor_scalar_mul(ot, pt[:, :D], r)
            nc.sync.dma_start(out=o_d[k * P : (k + 1) * P, :], in_=ot)

    return nc


_NC = None


def kernel(query, key, value, mask, Wq, bq, Wk, bk, Wv, bv):
    global _NC, LAST_RESULT
    bf16 = ml_dtypes.bfloat16
    B = query.shape[0]
    assert B == 8

    if _NC is None:
        _NC = _build()
        _NC.finalize()  # run bacc passes (wait splitting, reg alloc, ACT tables)

    wqk = np.ascontiguousarray(
        np.concatenate([np.asarray(Wq), np.asarray(Wk)], axis=1).astype(bf16)
    )
    wvv = np.ascontiguousarray(
        np.concatenate([np.asarray(Wv), np.asarray(Wv)], axis=1).astype(bf16)
    )
    bqk = np.concatenate([np.asarray(bq), np.asarray(bk)]).astype(np.float32)
    bvv = np.concatenate([np.asarray(bv), np.asarray(bv)]).astype(np.float32)

    in_maps = []
    for b in range(B):
        mb = ((np.asarray(mask[b], np.float32) - 1.0) * 1e9).reshape(NT, P).T
        in_maps.append(
            {
                "qT": np.ascontiguousarray(np.asarray(query[b]).T.astype(bf16)),
                "kT": np.ascontiguousarray(np.asarray(key[b]).T.astype(bf16)),
                "vT": np.ascontiguousarray(np.asarray(value[b]).T.astype(bf16)),
                "wqk": wqk,
                "wvv": wvv,
                "bqk": bqk.reshape(P, 1),
                "bvv": bvv.reshape(P, 1),
                "mb": np.ascontiguousarray(mb),
            }
        )

    res = run_bass_kernel_spmd(
        _NC,
        in_maps,
        core_ids=list(range(8)),
        trace=bool(os.environ.get("KERNEL_TRACE")),
    )
    LAST_RESULT = res
    out = np.stack([np.asarray(res.results[i]["o"]) for i in range(B)])
    return out.astype(np.float32)
